# revision 1
# baseline (speedup 1.0000x reference)
"""Banded circular-bias attention on 8 TRN2 NeuronCores.

Problem: B=2, L=2048, H=16, D=64 attention with additive circular relative
position bias  -min(|q-k|, L-|q-k|)  and key masking (mask==0 -> -1e9).

Key observation: scores/sqrt(D) ~ N(0,1) while the bias reaches -1024, so
softmax weights vanish beyond |q-k|_circ ~ 64 (omitted terms < e^-55 of the
max).  The dense L x L attention collapses to a +-64 circular band.

K-blocks are SHIFTED by 64 vs the q-tiles: block t covers keys
[128t+64, 128t+192), whose +-64 band is exactly queries [128t, 128t+256).
Conversely q-tile qi's band is exactly blocks qi-1 and qi -- each output tile
needs only TWO accumulating matmuls, with no partial or padded slices.

Sharding: 32 (batch, head) pairs -> 4 per core (2 heads x 2 batches).

Per-core kernel (bf16 matmul inputs, fp32 PSUM accumulation / output):
  - Q^T: [64, 2048+128] bf16, right wrap-halo (d on partitions)
  - K^T: [64, 2048+64] bf16, right wrap-halo
  - V_aug: [2048, 65] bf16, rows rolled by -64 to match the shifted blocks
    (ones column -> fused softmax denominator)
  - S^T per block t: matmul(lhsT=K^T[:, 128t+64:128t+192],
    rhs=Q^T[:, 128t:128t+256]) -> quarter of a [128, 4, 256] 2-bank slot;
    q - k = j - 64 - k_rel independent of t: ONE resident exp(bias) tile.
  - The key mask lives in V: masked rows of V_aug (incl. the ones column) are
    zeroed host-side, so masked keys add 0 to numerator AND denominator --
    exactly equivalent to the reference's -1e9 score replacement.
  - E = Exp(S^T/8) over FOUR blocks in one wide ScalarE op (PSUM->SBUF bf16,
    [128,4,256] from a 2-bank PSUM slot); P = E * exp(bias_band) in one wide
    4x DVE op.
  - O for a QUAD of qtiles shares one PSUM bank [128, 4, 65] = 1040B: two
    matmuls per qtile (blocks qi-1, qi): lhsT=PT slice [128k, 128q], rhs=V_aug
    block [128k, 65]; col 64 = denominator.  Per quad: ONE reciprocal [128,4],
    ONE stride-0-broadcast multiply [128,4,64] (evict+normalize), ONE DMA.
No transposes, no collectives.  TimelineSim: ~40 us/core; CoreSim matches a
dense float64 reference at rel err 2.5e-3 (tolerance 2e-2).
"""

import json
import os
import sys

import numpy as np

sys.path.insert(0, "/opt/trn_rl_repo")


def _fix_multiwaits(j):
    """The walrus in this container accepts at most ONE semaphore wait per
    instruction, but Tile's scheduler attaches several.  Hoist extra on_wait
    entries into standalone EventSemaphore instructions immediately before on
    the same engine queue (queues execute in order, so this is equivalent);
    same for extra on_update entries, hoisted to just after."""
    nw = nu = 0
    for f in j["functions"]:
        for bb in f["blocks"]:
            out = []
            for ins in bb["instructions"]:
                si = ins.get("sync_info") or {}
                waits = si.get("on_wait") or []
                if len(waits) > 1:
                    for w in waits[:-1]:
                        out.append({
                            "debug": ins.get("debug", 0),
                            "engine": ins["engine"],
                            "ins": [],
                            "name": f"hw{nw}_{ins['name']}",
                            "opcode": "EventSemaphore",
                            "outs": [],
                            "sync_info": {"on_update": [], "on_wait": [w]},
                        })
                        nw += 1
                    si["on_wait"] = [waits[-1]]
                out.append(ins)
                upds = si.get("on_update") or []
                if len(upds) > 1:
                    out.append({
                        "debug": ins.get("debug", 0),
                        "engine": ins["engine"],
                        "ins": [],
                        "name": f"hu{nu}_{ins['name']}",
                        "opcode": "EventSemaphore",
                        "outs": [],
                        "sync_info": {"on_update": upds[1:], "on_wait": []},
                    })
                    nu += 1
                    si["on_update"] = [upds[0]]
            bb["instructions"] = out
    return nw, nu


def _patch_nc(nc):
    orig = nc.to_json_bytes

    def patched(*a, **k):
        j = json.loads(orig(*a, **k))
        _fix_multiwaits(j)
        return json.dumps(j).encode()

    nc.to_json_bytes = patched
    return nc

B = 2
L = 2048
H = 16
D = 64
NCORES = 8
HPC = H // NCORES  # heads per core
PAIRS = B * HPC  # (b,h) pairs per core
NKT = L // 128  # 16 k-tiles
BAND = 256  # each shifted k-block [128t+64,128t+192) sees q in [128t,128t+256)
HALO = 128  # right-side wrap halo on Q^T; K^T carries a 64-col wrap halo
KHALO = 64

_CACHE = {}


def _build_nc():
    import concourse.bass as bass
    import concourse.mybir as mybir
    from concourse.tile import TileContext

    f32 = mybir.dt.float32
    bf16 = mybir.dt.bfloat16
    nc = bass.Bass()

    qt_ext = nc.declare_dram_parameter("qt", [PAIRS, 64, L + HALO], bf16, isOutput=False)
    kt_ext = nc.declare_dram_parameter("kt", [PAIRS, 64, L + KHALO], bf16, isOutput=False)
    va_ext = nc.declare_dram_parameter("va", [PAIRS, L, 65], bf16, isOutput=False)
    bb_ext = nc.declare_dram_parameter("bb", [128, 4, BAND], bf16, isOutput=False)
    out_ext = nc.declare_dram_parameter("out", [PAIRS, L, D], f32, isOutput=True)

    with TileContext(nc) as tc:
        with (
            tc.tile_pool(name="consts", bufs=1) as consts,
            tc.tile_pool(name="io", bufs=3) as io_pool,
            tc.tile_pool(name="pt", bufs=NKT // 4 + 2) as pt_pool,
            tc.tile_pool(name="work", bufs=3) as work,
            tc.tile_pool(name="psum_s", bufs=2, space="PSUM") as psum_s,
            tc.tile_pool(name="psum_o", bufs=3, space="PSUM") as psum_o,
        ):
            # EB2 = exp(band bias), duplicated side by side for paired k-tiles
            eb4_sb = consts.tile([128, 4, BAND], bf16)
            nc.sync.dma_start(eb4_sb, bb_ext[:, :, :])

            for p in range(PAIRS):
                b = p // HPC
                qt_sb = io_pool.tile([64, L + HALO], bf16, tag="qt")
                nc.sync.dma_start(qt_sb, qt_ext[p])
                kt_sb = io_pool.tile([64, L + KHALO], bf16, tag="kt")
                nc.sync.dma_start(kt_sb, kt_ext[p])
                va_sb = io_pool.tile([128, NKT, 65], bf16, tag="va")
                nc.sync.dma_start(
                    va_sb, va_ext[p].rearrange("(t kp) c -> kp t c", kp=128)
                )

                pt4s = [None] * (NKT // 4)

                ps4s = [None]

                def phase1(t, pt4s=pt4s, ps4s=ps4s, qt_sb=qt_sb, kt_sb=kt_sb):
                    # S^T for shifted k-block t (k in [128t+64, 128t+192)):
                    # [128 k, 256 q] with q in [128t, 128t+256).  Four blocks
                    # pack one 2-bank PSUM slot (256 f32 = 1KB per block).
                    if t % 4 == 0:
                        ps4 = psum_s.tile([128, 4, BAND], f32, tag="ps")
                        ps4s[0] = ps4
                    ps4 = ps4s[0]
                    nc.tensor.matmul(
                        ps4[:, t % 4, :],
                        kt_sb[:, t * 128 + 64 : t * 128 + 192],
                        qt_sb[:, t * 128 : t * 128 + BAND],
                        start=True,
                        stop=True,
                    )
                    # E = exp(S/8) for all four blocks in one op (mask lives
                    # in va); P = E * exp(bias) in one wide SBUF 4x op
                    if t % 4 == 3:
                        pt4 = pt_pool.tile([128, 4, BAND], bf16, tag="pt")
                        pt4s[t // 4] = pt4
                        nc.scalar.activation(
                            pt4,
                            ps4[:, :, :],
                            mybir.ActivationFunctionType.Exp,
                            bias=0.0,
                            scale=0.125,
                        )
                        nc.vector.tensor_mul(pt4, pt4, eb4_sb)

                def phase2quad(qi, pt4s=pt4s, va_sb=va_sb, p=p):
                    # qtiles qi..qi+3 share one PSUM bank: [128, 4, 65] = 1040B.
                    # band of qtile q = shifted blocks q-1 and q exactly
                    po4 = psum_o.tile([128, 4, 65], f32, tag="po")
                    for half in range(4):
                        q = qi + half
                        for ji, u in enumerate([q - 1, q]):
                            ui = u % NKT
                            j0 = (q - u) * 128  # 128, 0
                            nc.tensor.matmul(
                                po4[:, half, :],
                                pt4s[ui // 4][:, ui % 4, j0 : j0 + 128],
                                va_sb[:, ui, :],
                                start=(ji == 0),
                                stop=(ji == 1),
                            )
                    rec = work.tile([128, 4, 1], f32, tag="denom")
                    nc.vector.reciprocal(rec, po4[:, :, 64:65])
                    o_sb = work.tile([128, 4, 64], f32, tag="o")
                    src_ap, rec_ap = bass.broadcast_tensor_aps(
                        po4[:, :, 0:64], rec[:, :, :]
                    )
                    nc.vector.tensor_tensor(o_sb, src_ap, rec_ap, mybir.AluOpType.mult)
                    nc.sync.dma_start(
                        out_ext[p, qi * 128 : qi * 128 + 512, :].rearrange(
                            "(four qp) d -> qp four d", qp=128
                        ),
                        o_sb,
                    )

                # qtile qi needs shifted blocks qi-1 and qi; EB-mult on odd t.
                # quad (4k..4k+3) ready once block 4k+3 is done (t = 4k+3);
                # quad (0..3) needs block 15 (wrap) so it goes last.
                for t in range(NKT):
                    phase1(t)
                    if t % 4 == 3 and t >= 7:
                        phase2quad(t - 3)
                phase2quad(0)

    return _patch_nc(nc)


def _prep_in_maps(query_states, key_states, value_states, mask):
    import ml_dtypes

    bf16 = ml_dtypes.bfloat16
    q = np.ascontiguousarray(query_states, dtype=np.float32).reshape(B, L, H, D)
    k = np.ascontiguousarray(key_states, dtype=np.float32).reshape(B, L, H, D)
    v = np.ascontiguousarray(value_states, dtype=np.float32).reshape(B, L, H, D)


    # multiplicative band bias exp(-|q-k|), duplicated for paired k-tiles
    jj = np.arange(BAND)[None, :]
    kk = np.arange(128)[:, None]
    eb_band = np.exp(-np.abs(jj - 64 - kk).astype(np.float32))
    bb = np.ascontiguousarray(
        np.broadcast_to(eb_band[:, None, :], (128, 4, BAND))
    ).astype(bf16)

    in_maps = []
    for c in range(NCORES):
        pairs = [(bb_, 2 * c + hh) for bb_ in range(B) for hh in range(HPC)]
        qt = np.empty((PAIRS, 64, L + HALO), bf16)
        kt = np.empty((PAIRS, 64, L + KHALO), bf16)
        va = np.empty((PAIRS, L, 65), bf16)
        for i, (bi, hi) in enumerate(pairs):
            qT = q[bi, :, hi, :].T.astype(bf16)  # [64, L]
            qt[i, :, :L] = qT
            qt[i, :, L:] = qT[:, :HALO]
            kT = k[bi, :, hi, :].T.astype(bf16)
            kt[i, :, :L] = kT
            kt[i, :, L:] = kT[:, :KHALO]
            # V rows rolled by -64 so block t's rows are V[(128t+64+kp) % L]
            vv = np.empty((L, 65), np.float32)
            vv[:, :64] = v[bi, :, hi, :]
            vv[:, 64] = 1.0
            vv[np.asarray(mask)[bi] == 0, :] = 0.0
            va[i] = np.roll(vv, -64, axis=0).astype(bf16)
        in_maps.append({"qt": qt, "kt": kt, "va": va, "bb": bb.copy()})
    return in_maps


def _run(in_maps, trace=False):
    from concourse.bass_utils import run_bass_kernel_spmd

    if "nc" not in _CACHE:
        _CACHE["nc"] = _build_nc()
    res = run_bass_kernel_spmd(
        _CACHE["nc"], in_maps, core_ids=list(range(NCORES)), trace=trace
    )
    return res


def kernel(query_states, key_states, value_states, mask):
    in_maps = _prep_in_maps(query_states, key_states, value_states, mask)
    res = _run(in_maps, trace=bool(os.environ.get("KERNEL_TRACE")))
    out = np.empty((B, L, H, D), np.float32)
    for c in range(NCORES):
        o = res.results[c]["out"]  # [PAIRS, L, 64]
        i = 0
        for bi in range(B):
            for hh in range(HPC):
                out[bi, :, 2 * c + hh, :] = o[i]
                i += 1
    if bool(os.environ.get("KERNEL_TRACE")):
        _CACHE["last_exec_time_ns"] = res.exec_time_ns
        _CACHE["last_res"] = res
    return out.reshape(B, L, H * D)



# revision 8
# speedup vs baseline: 1.4213x; 1.4213x over previous
"""Banded circular-bias attention on 8 TRN2 NeuronCores (v2).

Problem: B=2, L=2048, H=16, D=64 attention with additive circular relative
position bias  -min(|q-k|, L-|q-k|)  and key masking (mask==0 -> -1e9).

scores/sqrt(D) ~ N(0,1) while the bias reaches -1024, so softmax weights
vanish beyond |q-k|_circ ~ 16 (omitted terms < e^-10 of the max).  The dense
L x L attention collapses to a +-16 circular band.

K-blocks are SHIFTED by 16 vs the q-tiles: block t covers keys
[128t+16, 128t+144), whose +-16 band is exactly queries [128t, 128t+160).
Each q-tile needs exactly TWO accumulating matmuls (blocks t-1, t).

Sharding: 32 (batch, head) pairs -> 4 per core (2 heads x 2 batches).

Per-core kernel (fp16 matmul inputs, fp32 PSUM):
  - Q^T/8: [64, 2048+32] fp16 with wrap halo; K^T: [64, 2048+16] fp16.
  - V_aug: [128, 16, 65] fp16, row kp of block t = V[(128t+16+kp) % L];
    ones column 64 -> fused softmax denominator; masked rows zeroed host-side.
  - S^T slot (4 blocks): matmul -> PSUM [128, 4, 160] f32;
    E = exp(S) in ONE wide ScalarE op -> PT [128, 4, 256] fp16 (cols 0:160);
    PT cols 160:256 zeroed (gpsimd memset) so the shifted phase-2 lhsT slice
    [128:256] reads exact zeros for queries that don't touch block t-1.
  - P = E * exp(-|dist|) in one wide DVE op; exp(-d) underflows to exact 0 in
    fp16 beyond d~17, so out-of-band entries die for free.
  - O: per q-tile two matmuls (blocks t-1, t) -> po PSUM [128, 16, 65] f32;
    per PAIR one reciprocal [128,16] + one broadcast multiply -> fp16 out,
    ONE contiguous output DMA per pair.
  - All input DMAs issued up-front (SBUF holds everything); emission is
    software-pipelined: pair p's phase 2 rides inside pair p+1's phase 1.
No transposes, no collectives.  All DMAs fully contiguous (>=2KB rows).
"""

import json
import os
import sys

import numpy as np

sys.path.insert(0, "/opt/trn_rl_repo")


def _fix_multiwaits(j):
    """The walrus in this container accepts at most ONE semaphore wait per
    instruction, but Tile's scheduler attaches several.  Hoist extra on_wait
    entries into standalone EventSemaphore instructions immediately before on
    the same engine queue (queues execute in order, so this is equivalent);
    same for extra on_update entries, hoisted to just after."""
    nw = nu = 0
    for f in j["functions"]:
        for bb in f["blocks"]:
            out = []
            for ins in bb["instructions"]:
                si = ins.get("sync_info") or {}
                waits = si.get("on_wait") or []
                if len(waits) > 1:
                    for w in waits[:-1]:
                        out.append({
                            "debug": ins.get("debug", 0),
                            "engine": ins["engine"],
                            "ins": [],
                            "name": f"hw{nw}_{ins['name']}",
                            "opcode": "EventSemaphore",
                            "outs": [],
                            "sync_info": {"on_update": [], "on_wait": [w]},
                        })
                        nw += 1
                    si["on_wait"] = [waits[-1]]
                out.append(ins)
                upds = si.get("on_update") or []
                if len(upds) > 1:
                    out.append({
                        "debug": ins.get("debug", 0),
                        "engine": ins["engine"],
                        "ins": [],
                        "name": f"hu{nu}_{ins['name']}",
                        "opcode": "EventSemaphore",
                        "outs": [],
                        "sync_info": {"on_update": upds[1:], "on_wait": []},
                    })
                    nu += 1
                    si["on_update"] = [upds[0]]
            bb["instructions"] = out
    return nw, nu


def _patch_nc(nc):
    orig = nc.to_json_bytes

    def patched(*a, **k):
        j = json.loads(orig(*a, **k))
        _fix_multiwaits(j)
        return json.dumps(j).encode()

    nc.to_json_bytes = patched
    return nc

B = 2
L = 2048
H = 16
D = 64
NCORES = 8
HPC = H // NCORES  # heads per core
PAIRS = B * HPC  # (b,h) pairs per core
NKT = L // 128  # 16 k-blocks
BAND = 16  # circular band half-width (e^-17 << tolerance)
W = 128 + 2 * BAND  # q-window per shifted k-block
QH = W - 128  # right wrap halo on Q^T
KH = BAND  # right wrap halo on K^T
NSL = 4  # blocks per PSUM slot
SLOTS = NKT // NSL

_CACHE = {}


def _build_nc():
    import concourse.bass as bass
    import concourse.mybir as mybir
    from concourse.tile import TileContext

    f32 = mybir.dt.float32
    f16 = mybir.dt.float16
    nc = bass.Bass()

    qt_ext = nc.declare_dram_parameter("qt", [PAIRS, 64, L + QH], f16, isOutput=False)
    kt_ext = nc.declare_dram_parameter("kt", [PAIRS, 64, L + KH], f16, isOutput=False)
    va_ext = nc.declare_dram_parameter("va", [PAIRS, 128, NKT, 65], f16, isOutput=False)
    eb_ext = nc.declare_dram_parameter("eb", [128, NSL, W], f16, isOutput=False)
    out_ext = nc.declare_dram_parameter("out", [PAIRS, 128, NKT, D], f16, isOutput=True)

    with TileContext(nc) as tc:
        with (
            tc.tile_pool(name="consts", bufs=1) as consts,
            tc.tile_pool(name="io", bufs=PAIRS) as io_pool,
            tc.tile_pool(name="pt", bufs=2 * SLOTS) as pt_pool,
            tc.tile_pool(name="work", bufs=2) as work,
            tc.tile_pool(name="psum_s", bufs=2, space="PSUM") as psum_s,
            tc.tile_pool(name="psum_o", bufs=1, space="PSUM") as psum_o,
        ):
            eb_sb = consts.tile([128, NSL, W], f16)
            nc.sync.dma_start(eb_sb, eb_ext[:, :, :])

            # all input DMAs up-front; everything stays resident in SBUF
            qts, kts, vas = [], [], []
            for p in range(PAIRS):
                qt_sb = io_pool.tile([64, L + QH], f16, tag="qt")
                nc.sync.dma_start(qt_sb, qt_ext[p])
                kt_sb = io_pool.tile([64, L + KH], f16, tag="kt")
                nc.sync.dma_start(kt_sb, kt_ext[p])
                va_sb = io_pool.tile([128, NKT, 65], f16, tag="va")
                nc.sync.dma_start(va_sb, va_ext[p])
                qts.append(qt_sb)
                kts.append(kt_sb)
                vas.append(va_sb)

            # pt slot tiles per pair, by slot index
            pts = {}
            pos = {}

            def phase1_slot(p, k):
                # S^T for shifted blocks 4k..4k+3 into one PSUM slot, then
                # E = exp(S) -> PT cols 0:160, PT cols 160:256 stay zero.
                # Block pitch padded to 256 f32 so no matmul output region
                # crosses a 2KB PSUM bank boundary.
                ps = psum_s.tile([128, NSL, 256], f32, tag="ps")
                for g in range(NSL):
                    t = NSL * k + g
                    nc.tensor.matmul(
                        ps[:, g, 0:W],
                        kts[p][:, t * 128 + BAND : t * 128 + BAND + 128],
                        qts[p][:, t * 128 : t * 128 + W],
                        start=True,
                        stop=True,
                    )
                pt = pt_pool.tile([128, NSL, 256], f16, tag="pt")
                pts[(p, k)] = pt
                nc.vector.memset(pt[:, :, W:256], 0.0)
                nc.scalar.activation(
                    pt[:, :, 0:W],
                    ps[:, :, 0:W],
                    mybir.ActivationFunctionType.Exp,
                    bias=0.0,
                    scale=1.0,
                )
                nc.vector.tensor_mul(pt[:, :, 0:W], pt[:, :, 0:W], eb_sb)

            def phase2_quad(p, k):
                # q-tiles 4k..4k+3 -> po[:, q, :]; band of q-tile t is blocks
                # t-1 (PT cols 128:256, zero-padded) and t (cols 0:128).
                po = pos[p]
                for g in range(NSL):
                    t = NSL * k + g
                    u = (t - 1) % NKT
                    nc.tensor.matmul(
                        po[:, t, 0:65],
                        pts[(p, k)][:, g, 0:128],
                        vas[p][:, t, :],
                        start=True,
                        stop=False,
                    )
                    nc.tensor.matmul(
                        po[:, t, 0:65],
                        pts[(p, u // NSL)][:, u % NSL, 128:256],
                        vas[p][:, u, :],
                        start=False,
                        stop=True,
                    )

            def norm_out(p):
                po = pos[p]
                rec = work.tile([128, NKT, 1], f32, tag="rec")
                nc.vector.reciprocal(rec, po[:, :, 64:65])
                o_sb = work.tile([128, NKT, D], f16, tag="o")
                src_ap, rec_ap = bass.broadcast_tensor_aps(po[:, :, 0:64], rec)
                nc.vector.tensor_tensor(o_sb, src_ap, rec_ap, mybir.AluOpType.mult)
                nc.sync.dma_start(out_ext[p], o_sb)

            # software pipeline: pair p's phase 2 rides inside pair p+1's
            # phase 1 so the PE queue never blocks on exp/eb latency.
            prev = None
            for p in range(PAIRS):
                # q-tile pitch padded to 128 f32 so each [*, t, 0:65] region
                # stays inside one 2KB PSUM bank.
                pos[p] = psum_o.tile([128, NKT, 128], f32, tag="po", name="po")
                for k in range(SLOTS):
                    phase1_slot(p, k)
                    if prev is not None:
                        if k < SLOTS - 1:
                            phase2_quad(prev, k + 1)
                        else:
                            phase2_quad(prev, 0)
                            norm_out(prev)
                prev = p
            for k in range(1, SLOTS):
                phase2_quad(prev, k)
            phase2_quad(prev, 0)
            norm_out(prev)

    return _patch_nc(nc)


def _prep_in_maps(query_states, key_states, value_states, mask):
    q = np.ascontiguousarray(query_states, dtype=np.float32).reshape(B, L, H, D)
    k = np.ascontiguousarray(key_states, dtype=np.float32).reshape(B, L, H, D)
    v = np.ascontiguousarray(value_states, dtype=np.float32).reshape(B, L, H, D)
    mk = np.asarray(mask)

    # multiplicative band bias exp(-|q-k|) replicated over the 4 slot blocks
    jj = np.arange(W)[None, :]
    mm = np.arange(128)[:, None]
    ebm = np.exp(-np.abs(jj - BAND - mm).astype(np.float32)).astype(np.float16)
    eb = np.ascontiguousarray(np.broadcast_to(ebm[:, None, :], (128, NSL, W)))

    # V_aug row gather: block t row kp = key (128t + BAND + kp) % L
    kp = np.arange(128)[:, None]
    tt = np.arange(NKT)[None, :]
    gidx = (128 * tt + BAND + kp) % L  # [128, NKT]

    in_maps = []
    for c in range(NCORES):
        pairs = [(bb_, 2 * c + hh) for bb_ in range(B) for hh in range(HPC)]
        qt = np.empty((PAIRS, 64, L + QH), np.float16)
        kt = np.empty((PAIRS, 64, L + KH), np.float16)
        va = np.empty((PAIRS, 128, NKT, 65), np.float16)
        for i, (bi, hi) in enumerate(pairs):
            qT = (q[bi, :, hi, :].T / 8.0).astype(np.float16)  # [64, L]
            qt[i, :, :L] = qT
            qt[i, :, L:] = qT[:, :QH]
            kT = k[bi, :, hi, :].T.astype(np.float16)
            kt[i, :, :L] = kT
            kt[i, :, L:] = kT[:, :KH]
            vv = np.empty((L, 65), np.float32)
            vv[:, :64] = v[bi, :, hi, :]
            vv[:, 64] = 1.0
            vv[mk[bi] == 0, :] = 0.0
            va[i] = vv[gidx].astype(np.float16)  # [128, NKT, 65]
        in_maps.append({"qt": qt, "kt": kt, "va": va, "eb": eb.copy()})
    return in_maps


def _run(in_maps, trace=False):
    from concourse.bass_utils import run_bass_kernel_spmd

    if "nc" not in _CACHE:
        _CACHE["nc"] = _build_nc()
    res = run_bass_kernel_spmd(
        _CACHE["nc"], in_maps, core_ids=list(range(NCORES)), trace=trace
    )
    return res


def kernel(query_states, key_states, value_states, mask):
    in_maps = _prep_in_maps(query_states, key_states, value_states, mask)
    res = _run(in_maps, trace=bool(os.environ.get("KERNEL_TRACE")))
    out = np.empty((B, L, H, D), np.float32)
    for c in range(NCORES):
        o = res.results[c]["out"]  # [PAIRS, 128, NKT, 64] fp16
        i = 0
        for bi in range(B):
            for hh in range(HPC):
                # out row 128*t + qp = o[i, qp, t, :]
                out[bi, :, 2 * c + hh, :] = (
                    o[i].astype(np.float32).transpose(1, 0, 2).reshape(L, D)
                )
                i += 1
    if bool(os.environ.get("KERNEL_TRACE")):
        _CACHE["last_exec_time_ns"] = res.exec_time_ns
        _CACHE["last_res"] = res
    return out.reshape(B, L, H * D)
